# revision 11
# baseline (speedup 1.0000x reference)
"""Decode-step GQA attention (bs=32, seq=1, 32 q heads / 8 kv heads, hd=128,
dim=4096, kv cache 2048) for 8 Trainium2 NeuronCores.

Sharding: tensor-parallel over heads. Core c owns kv head c and q heads
4c..4c+3: wq/wk/wv column-sharded, wo row-sharded, KV cache sharded on the
head axis. Each core computes a partial output projection; the host sums the
8 partials (no device collectives needed).

Device kernel layout choices:
  - K cache is staged host-side per core as K^T [b, hd, seq] so QK^T needs no
    on-device transpose; V stays natural [b, seq, hd].
  - scores are computed transposed ([seq, head] with seq on partitions) so
    exp runs on all 128 partitions; softmax denominators via a ones-vector
    matmul; normalization deferred to after the PV matmul where the
    denominator is a per-partition scalar.
  - The cache append (position start_pos) is handled by zeroing the stale
    position's exp weight and adding the new token's contribution as an extra
    PV accumulation term using a one-hot-masked outer product.
"""

import functools
import sys

import numpy as np

sys.path.insert(0, "/opt/trn_rl_repo")

import concourse.bass as bass  # noqa: E402
import concourse.tile as tile  # noqa: E402
from concourse import mybir  # noqa: E402
from concourse.bass_utils import run_bass_kernel_spmd  # noqa: E402

N_HEADS = 32
N_KV_HEADS = 8
HD = 128
DIM = 4096
BS = 32
MAXSEQ = 2048
NCORES = 8
HPC = N_HEADS // NCORES  # q heads per core (4)
QW = HPC * HD  # per-core wq width (512)
SCALE = 1.0 / float(np.sqrt(np.float32(HD)))

f32 = mybir.dt.float32


def _split_fat_waits(nc, max_waits=1):
    """walrus only encodes one semaphore wait per instruction; hoist extras
    onto preceding same-engine nops."""
    for f in nc.m.functions:
        for bb in f.blocks:
            new_list = []
            for ins in bb.instructions:
                si = ins.sync_info
                w = list(si.on_wait) if si and si.on_wait else []
                if len(w) > max_waits and ins.engine != mybir.EngineType.Unassigned:
                    extras, keep = w[:-max_waits], w[-max_waits:]
                    k = 0
                    while extras:
                        chunk, extras = extras[:max_waits], extras[max_waits:]
                        nop = mybir.InstNoOp(name=f"{ins.name}-wsplit{k}")
                        nop.engine = ins.engine
                        nop.sync_info = mybir.SyncInfo(on_wait=chunk, on_update=[])
                        new_list.append(nop)
                        k += 1
                    ins.sync_info.on_wait = keep
                new_list.append(ins)
            bb.instructions = new_list


def _build(start_pos):
    S = start_pos + 1  # attended sequence length
    NCH = (S + 127) // 128  # seq chunks
    LC = start_pos // 128  # chunk holding the appended position
    LP = start_pos % 128  # partition (within chunk) of the appended position

    nc = bass.Bass()
    xT = nc.declare_dram_parameter("xT", [128, DIM // 128, BS], f32, isOutput=False)
    wq = nc.declare_dram_parameter("wq", [DIM, QW], f32, isOutput=False)
    wk = nc.declare_dram_parameter("wk", [DIM, HD], f32, isOutput=False)
    wv = nc.declare_dram_parameter("wv", [DIM, HD], f32, isOutput=False)
    wo = nc.declare_dram_parameter("wo", [QW, DIM], f32, isOutput=False)
    kT = nc.declare_dram_parameter("kT", [BS, HD, MAXSEQ], f32, isOutput=False)
    v = nc.declare_dram_parameter("v", [BS, MAXSEQ, HD], f32, isOutput=False)
    cosq = nc.declare_dram_parameter("cosq", [BS, QW], f32, isOutput=False)
    sinq = nc.declare_dram_parameter("sinq", [BS, QW], f32, isOutput=False)
    cosk = nc.declare_dram_parameter("cosk", [BS, HD], f32, isOutput=False)
    sink = nc.declare_dram_parameter("sink", [BS, HD], f32, isOutput=False)
    iden = nc.declare_dram_parameter("iden", [128, 128], f32, isOutput=False)
    # smask[p] = 1.0 if stream position 128*LC+p is valid (< start_pos) else 0.0
    smask = nc.declare_dram_parameter("smask", [128, 1], f32, isOutput=False)
    out = nc.declare_dram_parameter("out", [BS, DIM], f32, isOutput=True)

    NKCH = DIM // 128  # contraction chunks for the projections (32)

    with tile.TileContext(nc) as tc:
        with (
            tc.tile_pool(name="const", bufs=1) as const,
            tc.tile_pool(name="wpool", bufs=4) as wpool,
            tc.tile_pool(name="ktpool", bufs=3) as ktpool,
            tc.tile_pool(name="vpool", bufs=3) as vpool,
            tc.tile_pool(name="exppool", bufs=2) as exppool,
            tc.tile_pool(name="small", bufs=2) as small,
            tc.tile_pool(name="wopool", bufs=4) as wopool,
            tc.tile_pool(name="outpool", bufs=1) as outpool,
        ):
            # ---- constants ----
            iden_sb = const.tile([128, 128], f32)
            nc.sync.dma_start(out=iden_sb[:], in_=iden[:])
            ones_sb = const.tile([128, 1], f32)
            nc.vector.memset(ones_sb[:], 1.0)
            xT_sb = const.tile([128, NKCH, BS], f32)
            nc.sync.dma_start(out=xT_sb[:], in_=xT[:])
            cosq_sb = const.tile([BS, QW], f32)
            nc.sync.dma_start(out=cosq_sb[:], in_=cosq[:])
            sinq_sb = const.tile([BS, QW], f32)
            nc.sync.dma_start(out=sinq_sb[:], in_=sinq[:])
            cosk_sb = const.tile([BS, HD], f32)
            nc.sync.dma_start(out=cosk_sb[:], in_=cosk[:])
            sink_sb = const.tile([BS, HD], f32)
            nc.sync.dma_start(out=sink_sb[:], in_=sink[:])
            smask_sb = const.tile([128, 1], f32)
            nc.sync.dma_start(out=smask_sb[:], in_=smask[:])

            qT_all = const.tile([128, HPC * BS], f32)  # col = 32h + b
            attnT = const.tile([128, HPC * BS], f32)  # col = 32h + b
            vnew_pad = const.tile([128, HD], f32)
            e_new_pad = const.tile([128, HPC], f32)

            # ---- phase 1: QKV projections ----
            with tc.tile_pool(name="psum_p1", bufs=1, space="PSUM") as psum_p1:
                q_ps = psum_p1.tile([BS, QW], f32)
                k_ps = psum_p1.tile([BS, HD], f32)
                v_ps = psum_p1.tile([BS, HD], f32)
                for k in range(NKCH):
                    w_t = wpool.tile([128, QW + 2 * HD], f32)
                    r = slice(128 * k, 128 * (k + 1))
                    nc.sync.dma_start(out=w_t[:, :QW], in_=wq[r, :])
                    nc.sync.dma_start(out=w_t[:, QW : QW + HD], in_=wk[r, :])
                    nc.sync.dma_start(out=w_t[:, QW + HD :], in_=wv[r, :])
                    st = k == 0
                    sp = k == NKCH - 1
                    lhsT = xT_sb[:, k, :]
                    nc.tensor.matmul(q_ps[:], lhsT, w_t[:, :QW], start=st, stop=sp)
                    nc.tensor.matmul(
                        k_ps[:], lhsT, w_t[:, QW : QW + HD], start=st, stop=sp
                    )
                    nc.tensor.matmul(
                        v_ps[:], lhsT, w_t[:, QW + HD :], start=st, stop=sp
                    )

                # ---- phase 2: rope, transposes, new-token prep ----
                p2 = const  # single-use tiles, lifetime to end of kernel
                # rope(q)
                q_sw = p2.tile([BS, QW], f32)
                q_ps3 = q_ps[:].rearrange("p (i two) -> p i two", two=2)
                q_sw3 = q_sw[:].rearrange("p (i two) -> p i two", two=2)
                nc.vector.tensor_copy(out=q_sw3[:, :, 0], in_=q_ps3[:, :, 1])
                nc.vector.tensor_copy(out=q_sw3[:, :, 1], in_=q_ps3[:, :, 0])
                q_ro = p2.tile([BS, QW], f32)
                nc.vector.tensor_tensor(
                    q_ro[:], q_ps[:], cosq_sb[:], mybir.AluOpType.mult
                )
                nc.vector.tensor_tensor(
                    q_sw[:], q_sw[:], sinq_sb[:], mybir.AluOpType.mult
                )
                nc.vector.tensor_tensor(q_ro[:], q_ro[:], q_sw[:], mybir.AluOpType.add)
                # rope(k)
                k_sw = p2.tile([BS, HD], f32)
                k_ps3 = k_ps[:].rearrange("p (i two) -> p i two", two=2)
                k_sw3 = k_sw[:].rearrange("p (i two) -> p i two", two=2)
                nc.vector.tensor_copy(out=k_sw3[:, :, 0], in_=k_ps3[:, :, 1])
                nc.vector.tensor_copy(out=k_sw3[:, :, 1], in_=k_ps3[:, :, 0])
                k_ro = p2.tile([BS, HD], f32)
                nc.vector.tensor_tensor(
                    k_ro[:], k_ps[:], cosk_sb[:], mybir.AluOpType.mult
                )
                nc.vector.tensor_tensor(
                    k_sw[:], k_sw[:], sink_sb[:], mybir.AluOpType.mult
                )
                nc.vector.tensor_tensor(k_ro[:], k_ro[:], k_sw[:], mybir.AluOpType.add)
                # v_new (no rope)
                nc.vector.memset(vnew_pad[:], 0.0)
                nc.vector.tensor_copy(out=vnew_pad[:BS, :], in_=v_ps[:])

                # q^T assembly: qT_all[:, 32h + b] = q_ro[b, 128h + :]
                qT_v = qT_all[:].rearrange("p (h b) -> p h b", h=HPC)
                with tc.tile_pool(name="psum_t", bufs=2, space="PSUM") as psum_t:
                    for h in range(HPC):
                        ps_qt = psum_t.tile([128, BS], f32)
                        nc.tensor.transpose(
                            ps_qt[:], q_ro[:, 128 * h : 128 * (h + 1)], iden_sb[:BS, :BS]
                        )
                        nc.vector.tensor_copy(out=qT_v[:, h, :], in_=ps_qt[:])

                # s_new[b, h] = q_ro[b, 128h:] . k_ro[b, :]
                qk_new = p2.tile([BS, QW], f32)
                k_bc = k_ro[:, None, :].to_broadcast([BS, HPC, HD])
                nc.vector.tensor_tensor(
                    qk_new[:].rearrange("p (h d) -> p h d", h=HPC),
                    q_ro[:].rearrange("p (h d) -> p h d", h=HPC),
                    k_bc,
                    mybir.AluOpType.mult,
                )
                s_new = p2.tile([BS, HPC], f32)
                nc.vector.tensor_reduce(
                    out=s_new[:],
                    in_=qk_new[:].rearrange("p (h d) -> p h d", h=HPC),
                    axis=mybir.AxisListType.X,
                    op=mybir.AluOpType.add,
                )
                nc.vector.memset(e_new_pad[:], 0.0)
                nc.scalar.activation(
                    out=e_new_pad[:BS, :],
                    in_=s_new[:],
                    func=mybir.ActivationFunctionType.Exp,
                    scale=SCALE,
                )

            # ---- phase 3: per-batch attention ----
            with (
                tc.tile_pool(name="ps_sT", bufs=2, space="PSUM") as psA,
                tc.tile_pool(name="ps_out", bufs=2, space="PSUM") as psB,
                tc.tile_pool(name="ps_den", bufs=1, space="PSUM") as psD,
                tc.tile_pool(name="ps_spec", bufs=1, space="PSUM") as psE,
                tc.tile_pool(name="ps_dT", bufs=1, space="PSUM") as psF,
                tc.tile_pool(name="ps_at", bufs=1, space="PSUM") as psG,
            ):
                attnT_v = attnT[:].rearrange("p (h b) -> p h b", h=HPC)
                for b in range(BS):
                    kt_t = ktpool.tile([128, S], f32)
                    nc.sync.dma_start(out=kt_t[:], in_=kT[b, :, :S])
                    v_t = vpool.tile([128, NCH, HD], f32)
                    nc.sync.dma_start(
                        out=v_t[:],
                        in_=v[b, : NCH * 128, :].rearrange("(c p) d -> p c d", p=128),
                    )

                    ps_sT = psA.tile([128, HPC * NCH], f32)
                    qT_b = qT_v[:, :, b]
                    for c in range(NCH):
                        cw = min(128, S - 128 * c)
                        nc.tensor.matmul(
                            ps_sT[:cw, HPC * c : HPC * (c + 1)],
                            kt_t[:, 128 * c : 128 * c + cw],
                            qT_b,
                            start=True,
                            stop=True,
                        )
                    exp_t = exppool.tile([128, HPC * NCH], f32)
                    nc.scalar.activation(
                        out=exp_t[:],
                        in_=ps_sT[:],
                        func=mybir.ActivationFunctionType.Exp,
                        scale=SCALE,
                    )
                    # zero the stale appended position (and any tail garbage)
                    nc.vector.tensor_tensor(
                        exp_t[:, HPC * LC : HPC * (LC + 1)],
                        exp_t[:, HPC * LC : HPC * (LC + 1)],
                        smask_sb[:].to_broadcast([128, HPC]),
                        mybir.AluOpType.mult,
                    )

                    ps_den = psD.tile([1, HPC * NCH], f32)
                    nc.tensor.matmul(
                        ps_den[:], ones_sb[:], exp_t[:], start=True, stop=True
                    )
                    ps_spec = psE.tile([1, HPC], f32)
                    nc.tensor.matmul(
                        ps_spec[:],
                        iden_sb[:, b : b + 1],
                        e_new_pad[:],
                        start=True,
                        stop=True,
                    )
                    den4 = small.tile([1, HPC], f32)
                    nc.vector.tensor_reduce(
                        out=den4[:],
                        in_=ps_den[:].rearrange("p (c h) -> p h c", h=HPC),
                        axis=mybir.AxisListType.X,
                        op=mybir.AluOpType.add,
                    )
                    nc.vector.tensor_tensor(
                        den4[:], den4[:], ps_spec[:], mybir.AluOpType.add
                    )
                    ps_dT = psF.tile([HPC, 1], f32)
                    nc.tensor.transpose(ps_dT[:], den4[:], iden_sb[:1, :1])
                    inv4 = small.tile([HPC, 1], f32)
                    nc.vector.reciprocal(inv4[:], ps_dT[:])

                    emask = small.tile([128, HPC], f32)
                    nc.vector.tensor_tensor(
                        emask[:],
                        e_new_pad[:],
                        iden_sb[:, b : b + 1].to_broadcast([128, HPC]),
                        mybir.AluOpType.mult,
                    )

                    ps_out = psB.tile([HPC, HD], f32)
                    for c in range(NCH):
                        cw = min(128, S - 128 * c)
                        nc.tensor.matmul(
                            ps_out[:],
                            exp_t[:cw, HPC * c : HPC * (c + 1)],
                            v_t[:cw, c, :],
                            start=(c == 0),
                            stop=False,
                        )
                    nc.tensor.matmul(
                        ps_out[:], emask[:], vnew_pad[:], start=False, stop=True
                    )

                    attn_sb = small.tile([HPC, HD], f32)
                    nc.vector.tensor_scalar(
                        out=attn_sb[:],
                        in0=ps_out[:],
                        scalar1=inv4[:],
                        scalar2=None,
                        op0=mybir.AluOpType.mult,
                    )
                    ps_at = psG.tile([128, HPC], f32)
                    nc.tensor.transpose(ps_at[:], attn_sb[:], iden_sb[:HPC, :HPC])
                    nc.vector.tensor_copy(out=attnT_v[:, :, b], in_=ps_at[:])

                    # prefetch wo during the second half of the batch loop
                    if b == 16:
                        wo_tiles = []
                        for j in range(HPC):
                            wo_t = wopool.tile([128, DIM], f32)
                            nc.sync.dma_start(
                                out=wo_t[:], in_=wo[128 * j : 128 * (j + 1), :]
                            )
                            wo_tiles.append(wo_t)

            # ---- phase 4: output projection ----
            NO = 8  # n-chunks of DIM/NO=512 (fp32 moving-operand max)
            NW = DIM // NO
            out_sb = outpool.tile([BS, DIM], f32)
            with tc.tile_pool(name="ps_o", bufs=2, space="PSUM") as psO:
                for n in range(NO):
                    ps_o = psO.tile([BS, NW], f32)
                    ns = slice(NW * n, NW * (n + 1))
                    for j in range(HPC):
                        nc.tensor.matmul(
                            ps_o[:],
                            attnT_v[:, j, :],
                            wo_tiles[j][:, ns],
                            start=(j == 0),
                            stop=(j == HPC - 1),
                        )
                    nc.vector.tensor_copy(out=out_sb[:, ns], in_=ps_o[:])
                    nc.sync.dma_start(out=out[:, ns], in_=out_sb[:, ns])

    _split_fat_waits(nc)
    return nc


@functools.lru_cache(maxsize=4)
def _built(start_pos):
    return _build(start_pos)


def _host_prep(x, wq, wk, wv, wo, cache_k, cache_v, freqs_cos, freqs_sin, start_pos):
    x = np.ascontiguousarray(np.asarray(x, dtype=np.float32)).reshape(BS, DIM)
    wq = np.asarray(wq, dtype=np.float32)
    wk = np.asarray(wk, dtype=np.float32)
    wv = np.asarray(wv, dtype=np.float32)
    wo = np.asarray(wo, dtype=np.float32)
    cache_k = np.asarray(cache_k, dtype=np.float32)
    cache_v = np.asarray(cache_v, dtype=np.float32)
    cos = np.asarray(freqs_cos, dtype=np.float32).reshape(HD // 2)
    sin = np.asarray(freqs_sin, dtype=np.float32).reshape(HD // 2)

    # x^T chunks: xT[p, c, b] = x[b, 128c + p]
    xT = np.ascontiguousarray(x.reshape(BS, DIM // 128, 128).transpose(2, 1, 0))

    cosF = np.empty(HD, np.float32)
    cosF[0::2] = cos
    cosF[1::2] = cos
    sinF = np.empty(HD, np.float32)
    sinF[0::2] = -sin
    sinF[1::2] = sin
    cosq = np.ascontiguousarray(np.broadcast_to(np.tile(cosF, HPC), (BS, QW)))
    sinq = np.ascontiguousarray(np.broadcast_to(np.tile(sinF, HPC), (BS, QW)))
    cosk = np.ascontiguousarray(np.broadcast_to(cosF, (BS, HD)))
    sink = np.ascontiguousarray(np.broadcast_to(sinF, (BS, HD)))
    iden = np.eye(128, dtype=np.float32)
    lc = start_pos // 128
    smask = (128 * lc + np.arange(128) < start_pos).astype(np.float32).reshape(128, 1)
    smask = np.ascontiguousarray(smask)

    in_maps = []
    for c in range(NCORES):
        in_maps.append(
            {
                "xT": xT,
                "wq": np.ascontiguousarray(wq[:, QW * c : QW * (c + 1)]),
                "wk": np.ascontiguousarray(wk[:, HD * c : HD * (c + 1)]),
                "wv": np.ascontiguousarray(wv[:, HD * c : HD * (c + 1)]),
                "wo": np.ascontiguousarray(wo[QW * c : QW * (c + 1), :]),
                "kT": np.ascontiguousarray(cache_k[:, :, c, :].transpose(0, 2, 1)),
                "v": np.ascontiguousarray(cache_v[:, :, c, :]),
                "cosq": cosq,
                "sinq": sinq,
                "cosk": cosk,
                "sink": sink,
                "iden": iden,
                "smask": smask,
            }
        )
    return in_maps


def kernel(
    x,
    wq,
    wk,
    wv,
    wo,
    cache_k,
    cache_v,
    freqs_cos,
    freqs_sin,
    start_pos,
    _trace=False,
    **_unused,
):
    sp = int(start_pos)
    nc = _built(sp)
    in_maps = _host_prep(
        x, wq, wk, wv, wo, cache_k, cache_v, freqs_cos, freqs_sin, sp
    )
    res = run_bass_kernel_spmd(nc, in_maps, list(range(NCORES)), trace=_trace)
    acc = np.zeros((BS, DIM), np.float32)
    for i in range(NCORES):
        acc += res.results[i]["out"]
    out = acc.reshape(BS, 1, DIM)
    if _trace:
        return out, res
    return out
